# revision 2
# baseline (speedup 1.0000x reference)
"""MoE (8 experts, top-2) Trainium2 kernel.

Strategy (per spec sharding_hint): expert parallelism. The host computes the
(cheap) router — logits, softmax, top-2, renormalized combine weights — and
dispatches each token to the cores owning its two experts ("all-to-all token
dispatch by top-k expert id" done at the sharding step, since kernel() holds
the full inputs host-side). Core e runs the expert-e FFN over its gathered
tokens, capacity-padded so all 8 cores run one SPMD program:

    YT = W2[e]^T @ gelu(W1[e]^T @ XT + b1[e])        (feature-major layouts)

Matmuls run in float32r (fp32 storage, full PE rate for moving dim >= 256).
The host then scatter-adds  (Y + b2[e]) * combine  back into the output.
"""

import os
import sys

import numpy as np

for _p in ("/opt/trn_rl_repo", "/root/.axon_site/_ro/trn_rl_repo"):
    if os.path.isdir(_p) and _p not in sys.path:
        sys.path.insert(0, _p)

NUM_EXPERTS = 8
TOP_K = 2
B, S, H, I = 4, 4096, 1024, 4096
T = B * S
P = 128
NT = 512           # max token tile = moving free dim (fp32 max 512)
C_DEFAULT = 4352   # capacity per expert (seed-0 max count 4302), mult of 256


def _token_tiles(C):
    """Split C into tiles of 512 plus at most one trailing 256."""
    assert C % 256 == 0
    tiles, off = [], 0
    while C - off >= 512:
        tiles.append((off, 512))
        off += 512
    if C - off:
        tiles.append((off, 256))
        off = C
    return tiles
KH = H // P        # 8 contraction chunks for stage 1
KI = I // P        # 32 contraction chunks for stage 2

_built = {}        # C -> (nc, input names)


def _build(C, reps=1):
    import concourse.bacc as bacc
    import concourse.mybir as mybir
    import concourse.tile as tile
    from concourse._compat import get_trn_type

    f32 = mybir.dt.float32
    f32r = mybir.dt.float32r
    bf16 = mybir.dt.bfloat16
    GELU = mybir.ActivationFunctionType.Gelu

    nc = bacc.Bacc(
        get_trn_type() or "TRN2",
        target_bir_lowering=False,
        debug=False,
        enable_asserts=False,
    )
    xt = nc.dram_tensor("xt", [H, C], f32r, kind="ExternalInput").ap()
    w1 = nc.dram_tensor("w1", [H, I], f32r, kind="ExternalInput").ap()
    b1 = nc.dram_tensor("b1", [I], f32, kind="ExternalInput").ap()
    w2 = nc.dram_tensor("w2", [I, H], bf16, kind="ExternalInput").ap()
    ya = nc.dram_tensor("ya", [H, C], f32, kind="ExternalOutput").ap()
    yb = nc.dram_tensor("yb", [H, C], f32, kind="ExternalOutput").ap()

    tiles = _token_tiles(C)
    IH = I // 2         # 2048: i-range per half-phase
    KIH = KI // 2       # 16 stage-2 contraction chunks per half
    XS = 2              # x sub-tiles per token tile (k-chunks split 2x4)
    HS2 = 4             # h2 sub-tiles per token tile (k-chunks split 4x4)

    with tile.TileContext(nc) as tc:
        with (
            tc.tile_pool(name="dram", bufs=1, space="DRAM") as drampool,
            tc.tile_pool(name="bias", bufs=1) as bpool,
            tc.tile_pool(name="wp", bufs=2) as wp,
            tc.tile_pool(name="xp", bufs=3) as xp,
            tc.tile_pool(name="hsp", bufs=4) as hsp,
            tc.tile_pool(name="h2p", bufs=3) as h2p,
            tc.tile_pool(name="yp", bufs=3) as yp,
            tc.tile_pool(name="psp", bufs=8, space="PSUM") as psp,
        ):
            hta = drampool.tile([KIH, P, C], bf16, tag="hta")
            htb = drampool.tile([KIH, P, C], bf16, tag="htb")
            b1sb = bpool.tile([P, KI], f32)
            nc.sync.dma_start(b1sb[:], b1.rearrange("(ib p) -> p ib", p=P))

            w1r = w1.rearrange("(ko p) i -> p ko i", p=P)
            w2r = w2.rearrange("(ko p) o -> p ko o", p=P)

            for rep in range(reps):
              for half in range(2):
                ht = hta if half == 0 else htb
                yout = ya if half == 0 else yb

                # -- Phase 1x: HT[i,t] = gelu(W1[:,i]^T XT + b1[i]), i in half
                w1sb = wp.tile([P, KH, IH], f32r, tag="w")

                def _load_w1(lo, hi, half=half, w1sb=w1sb):
                    nc.sync.dma_start(
                        w1sb[:, :, lo:hi],
                        w1r[:, :, half * IH + lo: half * IH + hi],
                    )

                def _load_x(t, s, toff, tsz):
                    xst = xp.tile([P, KH // XS, tsz], f32r, tag="x",
                                  name=f"x_{rep}_{half}_{t}_{s}")
                    nc.sync.dma_start(
                        xst[:],
                        xt[(s * KH // XS) * P:((s + 1) * KH // XS) * P,
                           toff:toff + tsz].rearrange(
                            "(ko p) n -> p ko n", p=P
                        ),
                    )
                    return xst

                # first w1 column chunk, then t=0 x tiles, then the rest —
                # so PE starts after ~2MB of DMA, not ~20MB
                _load_w1(0, 128)
                xs0 = [_load_x(0, s, tiles[0][0], tiles[0][1])
                       for s in range(XS)]
                for lo, hi in ((128, 256), (256, 384), (384, 512),
                               (512, 768), (768, 1024), (1024, 1280),
                               (1280, 1664), (1664, 2048)):
                    _load_w1(lo, hi)
                for t, (toff, tsz) in enumerate(tiles):
                    xs = xs0 if t == 0 else [
                        _load_x(t, s, toff, tsz) for s in range(XS)]
                    for ibl in range(KIH):
                        ib = half * KIH + ibl
                        ps = psp.tile([P, tsz], f32, tag="ps",
                                      name=f"ps1_{rep}_{half}_{t}_{ibl}")
                        for k in range(KH):
                            nc.tensor.matmul(
                                ps[:],
                                lhsT=w1sb[:, k, ibl * P:(ibl + 1) * P],
                                rhs=xs[k // (KH // XS)][:, k % (KH // XS)],
                                start=(k == 0),
                                stop=(k == KH - 1),
                            )
                        hs = hsp.tile([P, tsz], bf16, tag="hs",
                                      name=f"hs_{rep}_{half}_{t}_{ibl}")
                        nc.scalar.activation(
                            hs[:], ps[:], GELU, bias=b1sb[:, ib:ib + 1]
                        )
                        nc.sync.dma_start(ht[ibl, :, toff:toff + tsz], hs[:])

                # -- Phase 2x: Y_half[o,t] = sum_{i in half} W2[i,o] HT[i,t]
                w2sb = wp.tile([P, KIH, H], bf16, tag="w")
                for c in range(4):
                    cw = KIH // 4
                    nc.sync.dma_start(
                        w2sb[:, c * cw:(c + 1) * cw],
                        w2r[:, half * KIH + c * cw: half * KIH + (c + 1) * cw],
                    )
                HC = KIH // HS2   # 4 chunks per sub-tile
                for t, (toff, tsz) in enumerate(tiles):
                    pss = []
                    for _ob in range(H // P):
                        pst = psp.tile([P, tsz], f32, tag="ps",
                                       name=f"ps_{rep}_{half}_{t}_{_ob}")
                        pss.append(pst)
                    for s in range(HS2):
                        h2t = h2p.tile([P, HC, tsz], bf16, tag="h2",
                                       name=f"h2_{rep}_{half}_{t}_{s}")
                        nc.sync.dma_start(
                            h2t[:],
                            ht[s * HC:(s + 1) * HC, :,
                               toff:toff + tsz].rearrange("ko p n -> p ko n"),
                        )
                        for ob in range(H // P):
                            for kk in range(HC):
                                nc.tensor.matmul(
                                    pss[ob][:],
                                    lhsT=w2sb[:, s * HC + kk,
                                              ob * P:(ob + 1) * P],
                                    rhs=h2t[:, kk],
                                    start=(s == 0 and kk == 0),
                                    stop=(s == HS2 - 1 and kk == HC - 1),
                                )
                            if s == HS2 - 1:
                                ys = yp.tile([P, tsz], f32, tag="y",
                                             name=f"y_{rep}_{half}_{t}_{ob}")
                                nc.vector.tensor_copy(ys[:], pss[ob][:])
                                nc.sync.dma_start(
                                    yout[ob * P:(ob + 1) * P,
                                         toff:toff + tsz],
                                    ys[:],
                                )
    nc.finalize()
    return nc


def _routing(hidden, router_w, router_b):
    """Top-2 routing, bit-matching the jax reference on CPU."""
    import jax
    import jax.numpy as jnp

    cpu = jax.local_devices(backend="cpu")[0]
    with jax.default_device(cpu):
        logits = jnp.einsum("bsh,he->bse", jnp.asarray(hidden),
                            jnp.asarray(router_w)) + jnp.asarray(router_b)
        probs = jax.nn.softmax(logits, axis=-1)
        tkp, tki = jax.lax.top_k(probs, TOP_K)
        tkp = tkp / jnp.sum(tkp, axis=-1, keepdims=True)
        tkp_np = np.asarray(tkp).reshape(T, TOP_K)
        tki_np = np.asarray(tki).reshape(T, TOP_K)
    return tkp_np, tki_np


def _prepare(inputs):
    """Routing + per-expert input maps. Returns (C, in_maps, idx_e, prob_e)."""
    import ml_dtypes

    hidden_states = np.ascontiguousarray(
        inputs["hidden_states"], dtype=np.float32
    )
    w1 = np.ascontiguousarray(inputs["w1"], dtype=np.float32)
    b1 = np.ascontiguousarray(inputs["b1"], dtype=np.float32)
    w2 = np.ascontiguousarray(inputs["w2"], dtype=np.float32)

    w2_bf = w2.astype(ml_dtypes.bfloat16)
    tkp, tki = _routing(hidden_states, inputs["router_w"], inputs["router_b"])
    x = hidden_states.reshape(T, H)

    idx_e, prob_e = [], []
    for e in range(NUM_EXPERTS):
        hit = tki == e                       # [T, 2] bool
        idx = np.nonzero(hit.any(axis=1))[0]
        pe = np.where(hit[idx, 0], tkp[idx, 0], tkp[idx, 1]).astype(np.float32)
        idx_e.append(idx)
        prob_e.append(pe)

    maxn = max(len(ix) for ix in idx_e)
    C = C_DEFAULT if maxn <= C_DEFAULT else ((maxn + 255) // 256) * 256

    in_maps = []
    for e in range(NUM_EXPERTS):
        ix = idx_e[e]
        xt = np.zeros((H, C), dtype=np.float32)
        xt[:, :len(ix)] = x[ix].T
        in_maps.append({
            "xt": xt,
            "w1": w1[e],
            "b1": b1[e],
            "w2": w2_bf[e],
        })
    return C, in_maps, idx_e, prob_e


def kernel(hidden_states, w1, b1, w2, b2, router_w, router_b):
    from concourse import bass_utils

    b2 = np.ascontiguousarray(b2, dtype=np.float32)
    C, in_maps, idx_e, prob_e = _prepare({
        "hidden_states": hidden_states, "w1": w1, "b1": b1, "w2": w2,
        "router_w": router_w, "router_b": router_b,
    })
    if C not in _built:
        _built[C] = _build(C)
    nc = _built[C]

    res = bass_utils.run_bass_kernel_spmd(
        nc, in_maps, core_ids=list(range(NUM_EXPERTS))
    ).results

    out = np.zeros((T, H), dtype=np.float32)
    for e in range(NUM_EXPERTS):
        ix = idx_e[e]
        y = (res[e]["ya"][:, :len(ix)] + res[e]["yb"][:, :len(ix)]).T
        out[ix] += (y + b2[e]) * prob_e[e][:, None]
    return out.reshape(B, S, H)



# revision 3
# speedup vs baseline: 1.5584x; 1.5584x over previous
"""MoE (8 experts, top-2) Trainium2 kernel — v2.

Strategy (per spec sharding_hint): expert parallelism. The host computes the
(cheap) router — logits, softmax, top-2, renormalized combine weights — and
dispatches each token to the cores owning its two experts ("all-to-all token
dispatch by top-k expert id" done at the sharding step, since kernel() holds
the full inputs host-side). Core e runs the expert-e FFN over its gathered
tokens, capacity-padded so all 8 cores run one SPMD program.

v2 layout (vs v1): everything bf16, the gelu intermediate h stays in SBUF
(no DRAM round-trip), and I is processed in Q=4 slices of 1024 so one
slice's h fits in SBUF; each slice emits a partial y the host sums. Within
a slice, token tiles are processed in groups of 4 so each loaded stationary
weight (128x128) serves 4 matmuls — amortizing the unoverlapped LDWEIGHTS
(~53-107ns/MM otherwise, measured as the main gap vs the cost model in v1):

    h_q[i,t] = gelu(W1[:,i]^T XT + b1[i]),  i in slice q   (psum f32)
    Yq[o,t]  = sum_{i in q} W2[i,o] h_q[i,t]               (psum f32)
"""

import os
import sys

import numpy as np

for _p in ("/opt/trn_rl_repo", "/root/.axon_site/_ro/trn_rl_repo"):
    if os.path.isdir(_p) and _p not in sys.path:
        sys.path.insert(0, _p)

NUM_EXPERTS = 8
TOP_K = 2
B, S, H, I = 4, 4096, 1024, 4096
T = B * S
P = 128
NT = 512           # max token tile
C_DEFAULT = 4352   # capacity per expert (seed-0 max count 4302), mult of 256
KH = H // P        # 8 contraction chunks for stage 1
Q = 4              # I-slices
IQ = I // Q        # 1024 i-values per slice
NB = IQ // P       # 8 i-blocks (and stage-2 k-chunks) per slice
NO = H // P        # 8 output blocks
G = 4              # token tiles per weight-reuse group (psum-bank bound)

_built = {}        # (C, reps) -> nc


def _token_tiles(C):
    """Split C into tiles of 512 plus at most one trailing 256."""
    assert C % 256 == 0
    tiles, off = [], 0
    while C - off >= 512:
        tiles.append((off, 512))
        off += 512
    if C - off:
        tiles.append((off, 256))
        off = C
    return tiles


def _build(C, reps=1):
    import concourse.bacc as bacc
    import concourse.mybir as mybir
    import concourse.tile as tile
    from concourse._compat import get_trn_type

    f32 = mybir.dt.float32
    bf16 = mybir.dt.bfloat16
    GELU = mybir.ActivationFunctionType.Gelu

    nc = bacc.Bacc(
        get_trn_type() or "TRN2",
        target_bir_lowering=False,
        debug=False,
        enable_asserts=False,
    )
    xt = nc.dram_tensor("xt", [H, C], bf16, kind="ExternalInput").ap()
    w1 = nc.dram_tensor("w1", [P, KH, I], bf16, kind="ExternalInput").ap()
    b1 = nc.dram_tensor("b1", [I], f32, kind="ExternalInput").ap()
    w2 = nc.dram_tensor("w2", [Q * NO, P, NB, P], bf16,
                        kind="ExternalInput").ap()
    y = nc.dram_tensor("y", [Q, H, C], bf16, kind="ExternalOutput").ap()

    tiles = _token_tiles(C)
    groups = [tiles[i:i + G] for i in range(0, len(tiles), G)]

    with tile.TileContext(nc) as tc:
        with (
            tc.tile_pool(name="bias", bufs=1) as bpool,
            tc.tile_pool(name="w1p", bufs=2) as w1p,
            tc.tile_pool(name="w2p", bufs=3) as w2p,
            tc.tile_pool(name="xp", bufs=2 * G) as xp,
            tc.tile_pool(name="hp", bufs=1) as hp,
            tc.tile_pool(name="yp", bufs=4) as yp,
            tc.tile_pool(name="psp", bufs=8, space="PSUM") as psp,
        ):
            b1sb = bpool.tile([P, I // P], f32)
            nc.sync.dma_start(b1sb[:], b1.rearrange("(ib p) -> p ib", p=P))

            for rep in range(reps):
                for q in range(Q):
                    w1sb = w1p.tile([P, KH, IQ], bf16, tag="w1",
                                    name=f"w1_{rep}_{q}")
                    nc.sync.dma_start(w1sb[:], w1[:, :, q * IQ:(q + 1) * IQ])
                    h = hp.tile([P, NB, C], bf16, tag="h",
                                name=f"h_{rep}_{q}")

                    # -- stage 1: h = gelu(w1q^T x + b1q) over this I-slice
                    for g, tg in enumerate(groups):
                        xs = []
                        for ti, (toff, tsz) in enumerate(tg):
                            xst = xp.tile([P, KH, tsz], bf16, tag="x",
                                          name=f"x_{rep}_{q}_{g}_{ti}")
                            nc.sync.dma_start(
                                xst[:],
                                xt[:, toff:toff + tsz].rearrange(
                                    "(ko p) n -> p ko n", p=P),
                            )
                            xs.append(xst)
                        for ib in range(NB):
                            pss = [
                                psp.tile([P, tsz], f32, tag="ps",
                                         name=f"ps1_{rep}_{q}_{g}_{ib}_{ti}")
                                for ti, (toff, tsz) in enumerate(tg)
                            ]
                            for k in range(KH):
                                for ti in range(len(tg)):
                                    nc.tensor.matmul(
                                        pss[ti][:],
                                        lhsT=w1sb[:, k, ib * P:(ib + 1) * P],
                                        rhs=xs[ti][:, k],
                                        start=(k == 0),
                                        stop=(k == KH - 1),
                                    )
                            ibg = q * NB + ib
                            for ti, (toff, tsz) in enumerate(tg):
                                nc.scalar.activation(
                                    h[:, ib, toff:toff + tsz], pss[ti][:],
                                    GELU, bias=b1sb[:, ibg:ibg + 1],
                                )

                    # -- stage 2: y[q] = w2q^T h  (partial over this I-slice)
                    for ob in range(NO):
                        w2sb = w2p.tile([P, NB, P], bf16, tag="w2",
                                        name=f"w2_{rep}_{q}_{ob}")
                        nc.sync.dma_start(w2sb[:], w2[q * NO + ob])
                        for g, tg in enumerate(groups):
                            pss = [
                                psp.tile([P, tsz], f32, tag="ps",
                                         name=f"ps2_{rep}_{q}_{ob}_{g}_{ti}")
                                for ti, (toff, tsz) in enumerate(tg)
                            ]
                            for kk in range(NB):
                                for ti, (toff, tsz) in enumerate(tg):
                                    nc.tensor.matmul(
                                        pss[ti][:],
                                        lhsT=w2sb[:, kk],
                                        rhs=h[:, kk, toff:toff + tsz],
                                        start=(kk == 0),
                                        stop=(kk == NB - 1),
                                    )
                            for ti, (toff, tsz) in enumerate(tg):
                                ys = yp.tile([P, tsz], bf16, tag="y",
                                             name=f"y_{rep}_{q}_{ob}_{g}_{ti}")
                                nc.vector.tensor_copy(ys[:], pss[ti][:])
                                nc.sync.dma_start(
                                    y[q, ob * P:(ob + 1) * P,
                                      toff:toff + tsz],
                                    ys[:],
                                )
    nc.finalize()
    return nc


def _routing(hidden, router_w, router_b):
    """Top-2 routing, bit-matching the jax reference on CPU."""
    import jax
    import jax.numpy as jnp

    cpu = jax.local_devices(backend="cpu")[0]
    with jax.default_device(cpu):
        logits = jnp.einsum("bsh,he->bse", jnp.asarray(hidden),
                            jnp.asarray(router_w)) + jnp.asarray(router_b)
        probs = jax.nn.softmax(logits, axis=-1)
        tkp, tki = jax.lax.top_k(probs, TOP_K)
        tkp = tkp / jnp.sum(tkp, axis=-1, keepdims=True)
        tkp_np = np.asarray(tkp).reshape(T, TOP_K)
        tki_np = np.asarray(tki).reshape(T, TOP_K)
    return tkp_np, tki_np


def _prepare(inputs):
    """Routing + per-expert input maps. Returns (C, in_maps, idx_e, prob_e)."""
    import ml_dtypes

    bf16 = ml_dtypes.bfloat16
    hidden_states = np.ascontiguousarray(
        inputs["hidden_states"], dtype=np.float32
    )
    w1 = np.ascontiguousarray(inputs["w1"], dtype=np.float32)
    b1 = np.ascontiguousarray(inputs["b1"], dtype=np.float32)
    w2 = np.ascontiguousarray(inputs["w2"], dtype=np.float32)

    tkp, tki = _routing(hidden_states, inputs["router_w"], inputs["router_b"])
    x = hidden_states.reshape(T, H)

    idx_e, prob_e = [], []
    for e in range(NUM_EXPERTS):
        hit = tki == e                       # [T, 2] bool
        idx = np.nonzero(hit.any(axis=1))[0]
        pe = np.where(hit[idx, 0], tkp[idx, 0], tkp[idx, 1]).astype(np.float32)
        idx_e.append(idx)
        prob_e.append(pe)

    maxn = max(len(ix) for ix in idx_e)
    C = C_DEFAULT if maxn <= C_DEFAULT else ((maxn + 255) // 256) * 256

    # w1 packed [E, P, KH, I]: w1p[e, p, k, i] = w1[e, k*P+p, i]
    w1p = np.ascontiguousarray(
        w1.reshape(NUM_EXPERTS, KH, P, I).transpose(0, 2, 1, 3)
    ).astype(bf16)
    # w2 packed [E, Q*NO, P, NB, P]:
    #   w2p[e, q*NO+ob, p, kk, o'] = w2[e, q*IQ + kk*P + p, ob*P + o']
    w2p = np.ascontiguousarray(
        w2.reshape(NUM_EXPERTS, Q, NB, P, NO, P).transpose(0, 1, 4, 3, 2, 5)
        .reshape(NUM_EXPERTS, Q * NO, P, NB, P)
    ).astype(bf16)

    in_maps = []
    for e in range(NUM_EXPERTS):
        ix = idx_e[e]
        xt = np.zeros((H, C), dtype=bf16)
        xt[:, :len(ix)] = x[ix].T.astype(bf16)
        in_maps.append({
            "xt": xt,
            "w1": w1p[e],
            "b1": b1[e],
            "w2": w2p[e],
        })
    return C, in_maps, idx_e, prob_e


def kernel(hidden_states, w1, b1, w2, b2, router_w, router_b):
    from concourse import bass_utils

    b2 = np.ascontiguousarray(b2, dtype=np.float32)
    C, in_maps, idx_e, prob_e = _prepare({
        "hidden_states": hidden_states, "w1": w1, "b1": b1, "w2": w2,
        "router_w": router_w, "router_b": router_b,
    })
    if C not in _built:
        _built[C] = _build(C)
    nc = _built[C]

    res = bass_utils.run_bass_kernel_spmd(
        nc, in_maps, core_ids=list(range(NUM_EXPERTS))
    ).results

    out = np.zeros((T, H), dtype=np.float32)
    for e in range(NUM_EXPERTS):
        ix = idx_e[e]
        yq = res[e]["y"]                     # [Q, H, C] bf16
        yf = yq[:, :, :len(ix)].astype(np.float32).sum(axis=0)
        out[ix] += (yf.T + b2[e]) * prob_e[e][:, None]
    return out.reshape(B, S, H)


# revision 22
# speedup vs baseline: 1.5905x; 1.0206x over previous
"""MoE (8 experts, top-2) Trainium2 kernel — v2.

Strategy (per spec sharding_hint): expert parallelism. The host computes the
(cheap) router — logits, softmax, top-2, renormalized combine weights — and
dispatches each token to the cores owning its two experts ("all-to-all token
dispatch by top-k expert id" done at the sharding step, since kernel() holds
the full inputs host-side). Core e runs the expert-e FFN over its gathered
tokens, capacity-padded so all 8 cores run one SPMD program.

v2 layout (vs v1): everything bf16, the gelu intermediate h stays in SBUF
(no DRAM round-trip), and I is processed in Q=4 slices of 1024 so one
slice's h fits in SBUF; each slice emits a partial y the host sums. Within
a slice, token tiles are processed in groups of 4 so each loaded stationary
weight (128x128) serves 4 matmuls — amortizing the unoverlapped LDWEIGHTS
(~53-107ns/MM otherwise, measured as the main gap vs the cost model in v1):

    h_q[i,t] = gelu(W1[:,i]^T XT + b1[i]),  i in slice q   (psum f32)
    Yq[o,t]  = sum_{i in q} W2[i,o] h_q[i,t]               (psum f32)
"""

import os
import sys

import numpy as np

for _p in ("/opt/trn_rl_repo", "/root/.axon_site/_ro/trn_rl_repo"):
    if os.path.isdir(_p) and _p not in sys.path:
        sys.path.insert(0, _p)

NUM_EXPERTS = 8
TOP_K = 2
B, S, H, I = 4, 4096, 1024, 4096
T = B * S
P = 128
NT = 512           # max token tile
C_DEFAULT = 4224   # capacity per expert (seed-0 max count 4181), mult of 128
KH = H // P        # 8 contraction chunks for stage 1
Q = 4              # I-slices
IQ = I // Q        # 1024 i-values per slice
NB = IQ // P       # 8 i-blocks (and stage-2 k-chunks) per slice
NO = H // P        # 8 output blocks
G = 4              # token tiles per weight-reuse group (psum-bank bound)

_built = {}        # (C, reps) -> nc


def _token_tiles(C):
    """Split C into tiles of 512 plus at most one trailing 128/256/384."""
    assert C % 128 == 0
    tiles, off = [], 0
    while C - off >= 512:
        tiles.append((off, 512))
        off += 512
    if C - off:
        tiles.append((off, C - off))
        off = C
    return tiles


def _build(C, reps=1):
    import concourse.bacc as bacc
    import concourse.mybir as mybir
    import concourse.tile as tile
    from concourse._compat import get_trn_type

    f32 = mybir.dt.float32
    bf16 = mybir.dt.bfloat16
    GELU = mybir.ActivationFunctionType.Gelu

    nc = bacc.Bacc(
        get_trn_type() or "TRN2",
        target_bir_lowering=False,
        debug=False,
        enable_asserts=False,
    )
    xt = nc.dram_tensor("xt", [H, C], bf16, kind="ExternalInput").ap()
    w1 = nc.dram_tensor("w1", [P, KH, I], bf16, kind="ExternalInput").ap()
    b1 = nc.dram_tensor("b1", [I], f32, kind="ExternalInput").ap()
    w2 = nc.dram_tensor("w2", [Q * NO, P, NB, P], bf16,
                        kind="ExternalInput").ap()
    y = nc.dram_tensor("y", [Q, H, C], bf16, kind="ExternalOutput").ap()

    tiles = _token_tiles(C)
    groups = [tiles[i:i + G] for i in range(0, len(tiles), G)]

    with tile.TileContext(nc) as tc:
        with (
            tc.tile_pool(name="bias", bufs=1) as bpool,
            tc.tile_pool(name="w1p", bufs=1) as w1p,
            tc.tile_pool(name="w2p", bufs=2) as w2p,
            tc.tile_pool(name="xp", bufs=1) as xp,
            tc.tile_pool(name="hp", bufs=1) as hp,
            tc.tile_pool(name="yp", bufs=3) as yp,
            tc.tile_pool(name="psp", bufs=8, space="PSUM") as psp,
        ):
            b1sb = bpool.tile([P, I // P], f32)
            nc.sync.dma_start(b1sb[:], b1.rearrange("(ib p) -> p ib", p=P))

            for rep in range(reps):
                # x resident for the whole rep, loaded in tile-sized chunks
                # so the first group's compute starts after ~1MB of DMA.
                xall = xp.tile([P, KH, C], bf16, tag="xa", name=f"xa_{rep}")
                for toff, tsz in tiles:
                    nc.sync.dma_start(
                        xall[:, :, toff:toff + tsz],
                        xt[:, toff:toff + tsz].rearrange(
                            "(ko p) n -> p ko n", p=P),
                    )

                for q in range(Q):
                    w1sb = w1p.tile([P, KH, IQ], bf16, tag="w1",
                                    name=f"w1_{rep}_{q}")
                    nc.sync.dma_start(w1sb[:], w1[:, :, q * IQ:(q + 1) * IQ])
                    # all 8 o-block slices of this q-slice's w2, one DMA,
                    # prefetched during stage 1
                    w2sb = w2p.tile([P, NO, NB, P], bf16, tag="w2",
                                    name=f"w2_{rep}_{q}")
                    nc.sync.dma_start(
                        w2sb[:],
                        w2[q * NO:(q + 1) * NO].rearrange(
                            "a p b c -> p a b c"),
                    )
                    h = hp.tile([P, NB, C], bf16, tag="h",
                                name=f"h_{rep}_{q}")

                    # -- stage 1: h = gelu(w1q^T x + b1q) over this I-slice
                    for g, tg in enumerate(groups):
                        for ib in range(NB):
                            pss = [
                                psp.tile([P, tsz], f32, tag="ps",
                                         name=f"ps1_{rep}_{q}_{g}_{ib}_{ti}")
                                for ti, (toff, tsz) in enumerate(tg)
                            ]
                            # k_inner: stationary weight reused across the
                            # group's token tiles (LDWEIGHTS amortization
                            # beats psum-bank-switch cost; A/B-measured).
                            for k in range(KH):
                                for ti, (toff, tsz) in enumerate(tg):
                                    nc.tensor.matmul(
                                        pss[ti][:],
                                        lhsT=w1sb[:, k, ib * P:(ib + 1) * P],
                                        rhs=xall[:, k, toff:toff + tsz],
                                        start=(k == 0),
                                        stop=(k == KH - 1),
                                    )
                            ibg = q * NB + ib
                            for ti, (toff, tsz) in enumerate(tg):
                                nc.scalar.activation(
                                    h[:, ib, toff:toff + tsz], pss[ti][:],
                                    GELU, bias=b1sb[:, ibg:ibg + 1],
                                )

                    # -- stage 2: y[q] = w2q^T h  (partial over this I-slice)
                    for ob in range(NO):
                        for g, tg in enumerate(groups):
                            gstart = tg[0][0]
                            gsz = sum(tsz for _, tsz in tg)
                            pss = [
                                psp.tile([P, tsz], f32, tag="ps",
                                         name=f"ps2_{rep}_{q}_{ob}_{g}_{ti}")
                                for ti, (toff, tsz) in enumerate(tg)
                            ]
                            for kk in range(NB):
                                for ti, (toff, tsz) in enumerate(tg):
                                    nc.tensor.matmul(
                                        pss[ti][:],
                                        lhsT=w2sb[:, ob, kk],
                                        rhs=h[:, kk, toff:toff + tsz],
                                        start=(kk == 0),
                                        stop=(kk == NB - 1),
                                    )
                            # batch the group's drains into one SBUF tile,
                            # one DMA (on the ACT hwdge ring — SP carries
                            # x/w traffic)
                            yg = yp.tile([P, gsz], bf16, tag="y",
                                         name=f"y_{rep}_{q}_{ob}_{g}")
                            for ti, (toff, tsz) in enumerate(tg):
                                o = toff - gstart
                                nc.vector.tensor_copy(
                                    yg[:, o:o + tsz], pss[ti][:])
                            nc.scalar.dma_start(
                                y[q, ob * P:(ob + 1) * P,
                                  gstart:gstart + gsz],
                                yg[:],
                            )
    nc.finalize()
    return nc


def _routing(hidden, router_w, router_b):
    """Top-2 routing, bit-matching the jax reference on CPU."""
    import jax
    import jax.numpy as jnp

    cpu = jax.local_devices(backend="cpu")[0]
    with jax.default_device(cpu):
        logits = jnp.einsum("bsh,he->bse", jnp.asarray(hidden),
                            jnp.asarray(router_w)) + jnp.asarray(router_b)
        probs = jax.nn.softmax(logits, axis=-1)
        tkp, tki = jax.lax.top_k(probs, TOP_K)
        tkp = tkp / jnp.sum(tkp, axis=-1, keepdims=True)
        tkp_np = np.asarray(tkp).reshape(T, TOP_K)
        tki_np = np.asarray(tki).reshape(T, TOP_K)
    return tkp_np, tki_np


def _prepare(inputs):
    """Routing + per-expert input maps. Returns (C, in_maps, idx_e, prob_e)."""
    import ml_dtypes

    bf16 = ml_dtypes.bfloat16
    hidden_states = np.ascontiguousarray(
        inputs["hidden_states"], dtype=np.float32
    )
    w1 = np.ascontiguousarray(inputs["w1"], dtype=np.float32)
    b1 = np.ascontiguousarray(inputs["b1"], dtype=np.float32)
    w2 = np.ascontiguousarray(inputs["w2"], dtype=np.float32)

    tkp, tki = _routing(hidden_states, inputs["router_w"], inputs["router_b"])
    x = hidden_states.reshape(T, H)

    idx_e, prob_e = [], []
    for e in range(NUM_EXPERTS):
        hit = tki == e                       # [T, 2] bool
        idx = np.nonzero(hit.any(axis=1))[0]
        pe = np.where(hit[idx, 0], tkp[idx, 0], tkp[idx, 1]).astype(np.float32)
        idx_e.append(idx)
        prob_e.append(pe)

    maxn = max(len(ix) for ix in idx_e)
    C = C_DEFAULT if maxn <= C_DEFAULT else ((maxn + 127) // 128) * 128

    # w1 packed [E, P, KH, I]: w1p[e, p, k, i] = w1[e, k*P+p, i]
    w1p = np.ascontiguousarray(
        w1.reshape(NUM_EXPERTS, KH, P, I).transpose(0, 2, 1, 3)
    ).astype(bf16)
    # w2 packed [E, Q*NO, P, NB, P]:
    #   w2p[e, q*NO+ob, p, kk, o'] = w2[e, q*IQ + kk*P + p, ob*P + o']
    w2p = np.ascontiguousarray(
        w2.reshape(NUM_EXPERTS, Q, NB, P, NO, P).transpose(0, 1, 4, 3, 2, 5)
        .reshape(NUM_EXPERTS, Q * NO, P, NB, P)
    ).astype(bf16)

    in_maps = []
    for e in range(NUM_EXPERTS):
        ix = idx_e[e]
        xt = np.zeros((H, C), dtype=bf16)
        xt[:, :len(ix)] = x[ix].T.astype(bf16)
        in_maps.append({
            "xt": xt,
            "w1": w1p[e],
            "b1": b1[e],
            "w2": w2p[e],
        })
    return C, in_maps, idx_e, prob_e


def kernel(hidden_states, w1, b1, w2, b2, router_w, router_b):
    from concourse import bass_utils

    b2 = np.ascontiguousarray(b2, dtype=np.float32)
    C, in_maps, idx_e, prob_e = _prepare({
        "hidden_states": hidden_states, "w1": w1, "b1": b1, "w2": w2,
        "router_w": router_w, "router_b": router_b,
    })
    if C not in _built:
        _built[C] = _build(C)
    nc = _built[C]

    res = bass_utils.run_bass_kernel_spmd(
        nc, in_maps, core_ids=list(range(NUM_EXPERTS))
    ).results

    out = np.zeros((T, H), dtype=np.float32)
    for e in range(NUM_EXPERTS):
        ix = idx_e[e]
        yq = res[e]["y"]                     # [Q, H, C] bf16
        yf = yq[:, :, :len(ix)].astype(np.float32).sum(axis=0)
        out[ix] += (yf.T + b2[e]) * prob_e[e][:, None]
    return out.reshape(B, S, H)
